# revision 5
# baseline (speedup 1.0000x reference)
"""AtomicConv GNN message passing on 8 TRN2 NeuronCores.

Strategy: edges whose src atom type is not in features_to_use contribute
exactly zero (their one-hot row is all zeros), so they are dropped on the
host.  Remaining edges are sorted by dst and bucketed into 128-node
windows; each core owns a contiguous range of 98 windows (12544 nodes), so
per-core partial segment sums cover disjoint dst ranges and no collective
is needed.  On device, each core computes the K radial-basis weights per
edge with Scalar/Vector engines, expands them into a 48-wide (type, k)
message, builds a one-hot dst indicator per 128-edge tile with an
iota-compare, and scatters via TensorEngine matmuls accumulating in PSUM
per window.
"""

import sys

if "/opt/trn_rl_repo" not in sys.path:
    sys.path.insert(0, "/opt/trn_rl_repo")

import math

import numpy as np

N = 100_000
K = 8
T = 6
P = 128
NCORES = 8
NW = 98                   # 128-node windows per core
NPC = NW * P              # nodes per core (12544); 8*12544 = 100352 >= N
F32 = None                # filled lazily (mybir import deferred)

_graph_cache: dict = {}


def _build_graph(C: int, ic: np.ndarray, rm: np.ndarray, rs: np.ndarray):
    """Build + compile the SPMD per-core graph. C = 128-edge tiles per window."""
    from concourse import bacc, mybir
    import concourse.tile as tile

    f32 = mybir.dt.float32
    TILES = NW * C

    nc = bacc.Bacc("TRN2", target_bir_lowering=False, debug=False,
                   num_devices=NCORES)

    d_ext = nc.declare_dram_parameter("d", [P, TILES], f32, isOutput=False)
    rd_ext = nc.declare_dram_parameter("rd", [P, TILES], f32, isOutput=False)
    ty_ext = nc.declare_dram_parameter("ty", [P, TILES], f32, isOutput=False)
    o_ext = nc.declare_dram_parameter("o", [P, NW * 48], f32, isOutput=True)

    iota_np = np.broadcast_to(np.arange(P, dtype=np.float32), (P, P)).copy()
    iota_dram = nc.inline_tensor(iota_np.reshape(P, 1, P), name="iotac")
    trow_np = np.broadcast_to(np.arange(T, dtype=np.float32), (P, T)).copy()
    trow_dram = nc.inline_tensor(trow_np.reshape(P, 1, T), name="trowc")
    bias_np = np.broadcast_to((-rm).astype(np.float32), (P, K)).copy()
    bias_dram = nc.inline_tensor(bias_np, name="biasc")

    AF = mybir.ActivationFunctionType
    OP = mybir.AluOpType

    with tile.TileContext(nc) as tc:
        with tc.tile_pool(name="sb", bufs=1) as sb, \
             tc.tile_pool(name="sbw", bufs=2) as sbw, \
             tc.tile_pool(name="ps", bufs=2, space="PSUM") as ps:
            d_sb = sb.tile([P, TILES], f32)
            rd_sb = sb.tile([P, TILES], f32)
            ty_sb = sb.tile([P, TILES], f32)
            nc.sync.dma_start(out=d_sb[:], in_=d_ext[:])
            nc.sync.dma_start(out=rd_sb[:], in_=rd_ext[:])
            nc.sync.dma_start(out=ty_sb[:], in_=ty_ext[:])
            iota_sb = sb.tile([P, 1, P], f32)
            trow_sb = sb.tile([P, 1, T], f32)
            bias_sb = sb.tile([P, K], f32)
            nc.sync.dma_start(out=iota_sb[:], in_=iota_dram[:])
            nc.sync.dma_start(out=trow_sb[:], in_=trow_dram[:])
            nc.sync.dma_start(out=bias_sb[:], in_=bias_dram[:])

            # --- per-slot radial weights w[k] = exp(-g(d-r)^2) * (1 - sin^2(pi d / 2c)) * (d<=c)
            w_sb = sb.tile([P, TILES, K], f32)
            t_u = sb.tile([P, TILES], f32)
            t_e = sb.tile([P, TILES], f32)
            t_s = sb.tile([P, TILES], f32)
            t_h = sb.tile([P, TILES], f32)
            t_m = sb.tile([P, TILES], f32)
            for k in range(K):
                # u = (d - r_k)^2 ; e = exp(-g_k u)
                nc.scalar.activation(t_u[:], d_sb[:], AF.Square,
                                     bias=bias_sb[:, k:k + 1], scale=1.0)
                nc.scalar.activation(t_e[:], t_u[:], AF.Exp,
                                     scale=-float(rs[k]))
                # s = sin(pi min(d,c) / (2 c_k)) ; h = 1 - s^2 = 0.5*(cos(pi d/c_k)+1)
                # (the min keeps Sin's arg in [0, pi/2]; d>c slots are masked to 0)
                nc.vector.tensor_scalar(t_h[:], d_sb[:], float(ic[k]), None,
                                        OP.min)
                nc.scalar.activation(t_s[:], t_h[:], AF.Sin,
                                     scale=float(math.pi / (2.0 * ic[k])))
                nc.scalar.activation(t_s[:], t_s[:], AF.Square)
                nc.vector.tensor_scalar(t_h[:], t_s[:], -1.0, 1.0,
                                        OP.mult, OP.add)
                nc.vector.tensor_scalar(t_m[:], d_sb[:], float(ic[k]), None,
                                        OP.is_le)
                nc.vector.tensor_tensor(t_h[:], t_h[:], t_e[:], op=OP.mult)
                nc.vector.tensor_tensor(w_sb[:, :, k], t_h[:], t_m[:],
                                        op=OP.mult)

            out_sb = sb.tile([P, NW, 48], f32)

            for w in range(NW):
                ws = slice(w * C, (w + 1) * C)
                ind = sbw.tile([P, C, P], f32, tag="ind")
                nc.vector.tensor_tensor(
                    ind[:],
                    rd_sb[:, ws, None].to_broadcast([P, C, P]),
                    iota_sb[:].to_broadcast([P, C, P]),
                    op=OP.is_equal)
                oh = sbw.tile([P, C, T], f32, tag="oh")
                nc.vector.tensor_tensor(
                    oh[:],
                    ty_sb[:, ws, None].to_broadcast([P, C, T]),
                    trow_sb[:].to_broadcast([P, C, T]),
                    op=OP.is_equal)
                msg = sbw.tile([P, C, 48], f32, tag="msg")
                for t in range(T):
                    nc.vector.tensor_tensor(
                        msg[:, :, t * K:(t + 1) * K],
                        w_sb[:, ws, :],
                        oh[:, :, t:t + 1].to_broadcast([P, C, K]),
                        op=OP.mult)
                psum = ps.tile([P, 48], f32)
                for j in range(C):
                    nc.tensor.matmul(psum[:], lhsT=ind[:, j, :],
                                     rhs=msg[:, j, :],
                                     start=(j == 0), stop=(j == C - 1))
                nc.scalar.copy(out_sb[:, w, :], psum[:])

            nc.sync.dma_start(out=o_ext[:], in_=out_sb[:])

    nc.compile()
    return nc


def _prepare(feat, distances, src, dst, features_to_use, C_min=12):
    """Host-side filtering/sorting/padding into per-core regular grids."""
    ftu = np.asarray(features_to_use, dtype=np.float32)
    lut = np.full(32, -1, dtype=np.int32)
    for idx, v in enumerate(ftu):
        lut[int(round(float(v)))] = idx
    fi = np.clip(feat[:, 0].astype(np.int32), 0, 31)
    tsrc = lut[fi[src]]
    keep = tsrc >= 0
    d = np.ascontiguousarray(distances[keep, 0].astype(np.float32))
    dk = dst[keep].astype(np.int64)
    ty = tsrc[keep].astype(np.float32)

    order = np.argsort(dk, kind="stable")
    d = d[order]
    dk = dk[order]
    ty = ty[order]

    block = dk >> 7                      # global 128-node window id
    nblocks = NCORES * NW                # 784
    counts = np.bincount(block, minlength=nblocks)
    C = max(C_min, int(math.ceil(counts.max() / P)))
    starts = np.zeros(nblocks, dtype=np.int64)
    np.cumsum(counts[:-1], out=starts[1:])
    rank = np.arange(dk.size, dtype=np.int64) - starts[block]

    S = C * P
    D = np.full((NCORES, NW, S), 1.0e4, dtype=np.float32)
    RD = np.zeros((NCORES, NW, S), dtype=np.float32)
    TY = np.full((NCORES, NW, S), float(T), dtype=np.float32)
    core = (block // NW).astype(np.int64)
    blk = (block % NW).astype(np.int64)
    D[core, blk, rank] = d
    RD[core, blk, rank] = (dk & 127).astype(np.float32)
    TY[core, blk, rank] = ty

    # device layout: [partition p = slot%128, tile = w*C + j]
    def dev(a):
        return np.ascontiguousarray(
            a.reshape(NCORES, NW * C, P).transpose(0, 2, 1))

    return dev(D), dev(RD), dev(TY), C


def kernel(**inputs):
    feat = np.asarray(inputs["feat"], dtype=np.float32)
    distances = np.asarray(inputs["distances"], dtype=np.float32)
    src = np.asarray(inputs["src"]).astype(np.int64)
    dst = np.asarray(inputs["dst"]).astype(np.int64)
    ic = np.asarray(inputs["interaction_cutoffs"], dtype=np.float64)
    rm = np.asarray(inputs["rbf_kernel_means"], dtype=np.float64)
    rs = np.asarray(inputs["rbf_kernel_scaling"], dtype=np.float64)
    ftu = np.asarray(inputs["features_to_use"], dtype=np.float32)

    D, RD, TY, C = _prepare(feat, distances, src, dst, ftu)

    key = (C, ic.tobytes(), rm.tobytes(), rs.tobytes())
    nc = _graph_cache.get(key)
    if nc is None:
        nc = _build_graph(C, ic, rm, rs)
        _graph_cache.clear()
        _graph_cache[key] = nc

    from concourse.bass_utils import run_bass_kernel_spmd

    in_maps = [{"d": D[c], "rd": RD[c], "ty": TY[c]} for c in range(NCORES)]
    res = run_bass_kernel_spmd(nc, in_maps, core_ids=list(range(NCORES)),
                               trace=False)

    out = np.empty((NCORES * NPC, T * K), dtype=np.float32)
    for c in range(NCORES):
        oc = res.results[c]["o"].reshape(P, NW, 48)
        out[c * NPC:(c + 1) * NPC] = (
            oc.transpose(1, 0, 2).reshape(NPC, 48))
    return out[:N]


# revision 9
# speedup vs baseline: 15.2988x; 15.2988x over previous
"""AtomicConv GNN message passing on 8 TRN2 NeuronCores.

Strategy: edges whose src atom type is not in features_to_use contribute
exactly zero (their one-hot row is all zeros), so they are dropped on the
host.  Remaining edges are sorted by dst and bucketed into 128-node
windows; each core owns a contiguous range of 98 windows (12544 nodes), so
per-core partial segment sums cover disjoint dst ranges and no collective
is needed.  On device, each core computes the K radial-basis weights per
edge with Scalar/Vector engines, expands them into a 48-wide (type, k)
message, builds a one-hot dst indicator per 128-edge tile with an
iota-compare, and scatters via TensorEngine matmuls accumulating in PSUM
per window.
"""

import sys

if "/opt/trn_rl_repo" not in sys.path:
    sys.path.insert(0, "/opt/trn_rl_repo")

import math

import numpy as np

N = 100_000
K = 8
T = 6
P = 128
NCORES = 8
NW = 98                   # 128-node windows per core
NPC = NW * P              # nodes per core (12544); 8*12544 = 100352 >= N
F32 = None                # filled lazily (mybir import deferred)

_graph_cache: dict = {}


def _build_graph(C: int, ic: np.ndarray, rm: np.ndarray, rs: np.ndarray):
    """Build + compile the SPMD per-core graph. C = 128-edge tiles per window."""
    from concourse import bacc, mybir
    import concourse.tile as tile

    f32 = mybir.dt.float32
    TILES = NW * C

    nc = bacc.Bacc("TRN2", target_bir_lowering=False, debug=False,
                   num_devices=NCORES)

    d_ext = nc.declare_dram_parameter("d", [P, TILES], f32, isOutput=False)
    rd_ext = nc.declare_dram_parameter("rd", [P, TILES], f32, isOutput=False)
    ty_ext = nc.declare_dram_parameter("ty", [P, TILES], f32, isOutput=False)
    o_ext = nc.declare_dram_parameter("o", [P, NW * 48], f32, isOutput=True)

    iota_np = np.broadcast_to(np.arange(P, dtype=np.float32), (P, P)).copy()
    iota_dram = nc.inline_tensor(iota_np.reshape(P, 1, P), name="iotac")
    trow_np = np.broadcast_to(np.arange(T, dtype=np.float32), (P, T)).copy()
    trow_dram = nc.inline_tensor(trow_np.reshape(P, 1, T), name="trowc")
    bias_np = np.broadcast_to((-rm).astype(np.float32), (P, K)).copy()
    bias_dram = nc.inline_tensor(bias_np, name="biasc")

    AF = mybir.ActivationFunctionType
    OP = mybir.AluOpType

    with tile.TileContext(nc) as tc:
        with tc.tile_pool(name="sb", bufs=1) as sb, \
             tc.tile_pool(name="sbw", bufs=2) as sbw, \
             tc.tile_pool(name="ps", bufs=2, space="PSUM") as ps:
            d_sb = sb.tile([P, TILES], f32)
            rd_sb = sb.tile([P, TILES], f32)
            ty_sb = sb.tile([P, TILES], f32)
            nc.sync.dma_start(out=d_sb[:], in_=d_ext[:])
            nc.sync.dma_start(out=rd_sb[:], in_=rd_ext[:])
            nc.sync.dma_start(out=ty_sb[:], in_=ty_ext[:])
            iota_sb = sb.tile([P, 1, P], f32)
            trow_sb = sb.tile([P, 1, T], f32)
            bias_sb = sb.tile([P, K], f32)
            nc.sync.dma_start(out=iota_sb[:], in_=iota_dram[:])
            nc.sync.dma_start(out=trow_sb[:], in_=trow_dram[:])
            nc.sync.dma_start(out=bias_sb[:], in_=bias_dram[:])

            # --- per-slot radial weights w[k] = exp(-g(d-r)^2) * (1 - sin^2(pi d / 2c)) * (d<=c)
            w_sb = sb.tile([P, TILES, K], f32)
            t_u = sb.tile([P, TILES], f32)
            t_e = sb.tile([P, TILES], f32)
            t_s = sb.tile([P, TILES], f32)
            t_h = sb.tile([P, TILES], f32)
            t_m = sb.tile([P, TILES], f32)
            for k in range(K):
                # u = (d - r_k)^2 ; e = exp(-g_k u)
                nc.scalar.activation(t_u[:], d_sb[:], AF.Square,
                                     bias=bias_sb[:, k:k + 1], scale=1.0)
                nc.scalar.activation(t_e[:], t_u[:], AF.Exp,
                                     scale=-float(rs[k]))
                # s = sin(pi min(d,c) / (2 c_k)) ; h = 1 - s^2 = 0.5*(cos(pi d/c_k)+1)
                # (the min keeps Sin's arg in [0, pi/2]; d>c slots are masked to 0)
                nc.vector.tensor_scalar(t_h[:], d_sb[:], float(ic[k]), None,
                                        OP.min)
                nc.scalar.activation(t_s[:], t_h[:], AF.Sin,
                                     scale=float(math.pi / (2.0 * ic[k])))
                nc.scalar.activation(t_s[:], t_s[:], AF.Square)
                nc.vector.tensor_scalar(t_h[:], t_s[:], -1.0, 1.0,
                                        OP.mult, OP.add)
                nc.vector.tensor_scalar(t_m[:], d_sb[:], float(ic[k]), None,
                                        OP.is_le)
                nc.vector.tensor_tensor(t_h[:], t_h[:], t_e[:], op=OP.mult)
                nc.vector.tensor_tensor(w_sb[:, :, k], t_h[:], t_m[:],
                                        op=OP.mult)

            out_sb = sb.tile([P, NW, 48], f32)

            for w in range(NW):
                ws = slice(w * C, (w + 1) * C)
                ind = sbw.tile([P, C, P], f32, tag="ind")
                nc.vector.tensor_tensor(
                    ind[:],
                    rd_sb[:, ws, None].to_broadcast([P, C, P]),
                    iota_sb[:].to_broadcast([P, C, P]),
                    op=OP.is_equal)
                oh = sbw.tile([P, C, T], f32, tag="oh")
                nc.vector.tensor_tensor(
                    oh[:],
                    ty_sb[:, ws, None].to_broadcast([P, C, T]),
                    trow_sb[:].to_broadcast([P, C, T]),
                    op=OP.is_equal)
                msg = sbw.tile([P, C, 48], f32, tag="msg")
                for t in range(T):
                    nc.vector.tensor_tensor(
                        msg[:, :, t * K:(t + 1) * K],
                        w_sb[:, ws, :],
                        oh[:, :, t:t + 1].to_broadcast([P, C, K]),
                        op=OP.mult)
                psum = ps.tile([P, 48], f32)
                for j in range(C):
                    nc.tensor.matmul(psum[:], lhsT=ind[:, j, :],
                                     rhs=msg[:, j, :],
                                     start=(j == 0), stop=(j == C - 1))
                nc.scalar.copy(out_sb[:, w, :], psum[:])

            nc.sync.dma_start(out=o_ext[:], in_=out_sb[:])

    nc.compile()
    return nc


def _prepare(feat, distances, src, dst, features_to_use, C_min=12):
    """Host-side filtering/sorting/padding into per-core regular grids."""
    ftu = np.asarray(features_to_use, dtype=np.float32)
    lut = np.full(32, -1, dtype=np.int32)
    for idx, v in enumerate(ftu):
        lut[int(round(float(v)))] = idx
    fi = np.clip(feat[:, 0].astype(np.int32), 0, 31)
    tsrc = lut[fi[src]]
    keep = tsrc >= 0
    d = np.ascontiguousarray(distances[keep, 0].astype(np.float32))
    dk = dst[keep].astype(np.int64)
    ty = tsrc[keep].astype(np.float32)

    order = np.argsort(dk, kind="stable")
    d = d[order]
    dk = dk[order]
    ty = ty[order]

    block = dk >> 7                      # global 128-node window id
    nblocks = NCORES * NW                # 784
    counts = np.bincount(block, minlength=nblocks)
    C = max(C_min, int(math.ceil(counts.max() / P)))
    starts = np.zeros(nblocks, dtype=np.int64)
    np.cumsum(counts[:-1], out=starts[1:])
    rank = np.arange(dk.size, dtype=np.int64) - starts[block]

    S = C * P
    D = np.full((NCORES, NW, S), 1.0e4, dtype=np.float32)
    RD = np.zeros((NCORES, NW, S), dtype=np.float32)
    TY = np.full((NCORES, NW, S), float(T), dtype=np.float32)
    core = (block // NW).astype(np.int64)
    blk = (block % NW).astype(np.int64)
    D[core, blk, rank] = d
    RD[core, blk, rank] = (dk & 127).astype(np.float32)
    TY[core, blk, rank] = ty

    # device layout: [partition p = slot%128, tile = w*C + j]
    def dev(a):
        return np.ascontiguousarray(
            a.reshape(NCORES, NW * C, P).transpose(0, 2, 1))

    return dev(D), dev(RD), dev(TY), C


def _make_runner(nc):
    """Build the jitted 8-core shard_map executable ONCE (mirrors
    bass2jax.run_bass_via_pjrt's multi-core branch) so repeat calls pay only
    dispatch + device execution, not re-tracing."""
    import jax
    import numpy as _np
    from jax.experimental.shard_map import shard_map
    from jax.sharding import Mesh, PartitionSpec
    from concourse import bass2jax, mybir

    bass2jax.install_neuronx_cc_hook()
    partition_name = (nc.partition_id_tensor.name
                      if nc.partition_id_tensor else None)

    in_names, out_names, out_avals, zero_outs = [], [], [], []
    for alloc in nc.m.functions[0].allocations:
        if not isinstance(alloc, mybir.MemoryLocationSet):
            continue
        name = alloc.memorylocations[0].name
        if alloc.kind == "ExternalInput":
            if name != partition_name:
                in_names.append(name)
        elif alloc.kind == "ExternalOutput":
            out_names.append(name)
            shape = tuple(alloc.tensor_shape)
            dtype = mybir.dt.np(alloc.dtype)
            out_avals.append(jax.core.ShapedArray(shape, dtype))
            zero_outs.append(_np.zeros(shape, dtype))
    n_params = len(in_names)
    n_outs = len(out_avals)
    all_names = in_names + out_names
    if partition_name is not None:
        all_names = all_names + [partition_name]

    def _body(*args):
        operands = list(args)
        if partition_name is not None:
            operands.append(bass2jax.partition_id_tensor())
        outs = bass2jax._bass_exec_p.bind(
            *operands,
            out_avals=tuple(out_avals),
            in_names=tuple(all_names),
            out_names=tuple(out_names),
            lowering_input_output_aliases=(),
            sim_require_finite=True,
            sim_require_nnan=True,
            nc=nc,
        )
        return tuple(outs)

    devices = jax.devices()[:NCORES]
    mesh = Mesh(np.asarray(devices), ("core",))
    specs = (PartitionSpec("core"),) * (n_params + n_outs)
    sharded = jax.jit(
        shard_map(_body, mesh=mesh, in_specs=specs,
                  out_specs=(PartitionSpec("core"),) * n_outs,
                  check_rep=False),
        donate_argnums=tuple(range(n_params, n_params + n_outs)),
        keep_unused=True)

    def run(in_maps, device_args=None):
        if device_args is None:
            device_args = stage_inputs(in_maps)
        concat_zeros = [np.zeros((NCORES * z.shape[0], *z.shape[1:]), z.dtype)
                        for z in zero_outs]
        out_arrs = sharded(*device_args, *concat_zeros)
        return [
            {name: np.asarray(out_arrs[i]).reshape(
                NCORES, *out_avals[i].shape)[c]
             for i, name in enumerate(out_names)}
            for c in range(NCORES)
        ]

    def stage_inputs(in_maps):
        return [np.concatenate([np.asarray(in_maps[c][name])
                                for c in range(NCORES)], axis=0)
                for name in in_names]

    run.stage_inputs = stage_inputs
    run.sharded = sharded
    run.zero_outs = zero_outs
    return run


def kernel(**inputs):
    feat = np.asarray(inputs["feat"], dtype=np.float32)
    distances = np.asarray(inputs["distances"], dtype=np.float32)
    src = np.asarray(inputs["src"]).astype(np.int64)
    dst = np.asarray(inputs["dst"]).astype(np.int64)
    ic = np.asarray(inputs["interaction_cutoffs"], dtype=np.float64)
    rm = np.asarray(inputs["rbf_kernel_means"], dtype=np.float64)
    rs = np.asarray(inputs["rbf_kernel_scaling"], dtype=np.float64)
    ftu = np.asarray(inputs["features_to_use"], dtype=np.float32)

    D, RD, TY, C = _prepare(feat, distances, src, dst, ftu)

    key = (C, ic.tobytes(), rm.tobytes(), rs.tobytes())
    ent = _graph_cache.get(key)
    if ent is None:
        nc = _build_graph(C, ic, rm, rs)
        runner = _make_runner(nc)
        _graph_cache.clear()
        _graph_cache[key] = ent = (nc, runner)
    nc, runner = ent

    in_maps = [{"d": D[c], "rd": RD[c], "ty": TY[c]} for c in range(NCORES)]
    results = runner(in_maps)

    out = np.empty((NCORES * NPC, T * K), dtype=np.float32)
    for c in range(NCORES):
        oc = results[c]["o"].reshape(P, NW, 48)
        out[c * NPC:(c + 1) * NPC] = (
            oc.transpose(1, 0, 2).reshape(NPC, 48))
    return out[:N]
